# revision 20
# baseline (speedup 1.0000x reference)
"""GCN encoder (2-layer GCNConv) Trainium2 Bass kernel, 8-core SPMD.

out = A_hat @ relu(A_hat @ x @ W1 + b1) @ W2 + b2,  A_hat = D^-1/2 (A+I) D^-1/2

Strategy (1D graph partition by destination node):
 - nodes split into 8 contiguous ranges of 12500; each core owns its range's
   aggregations for both layers.
 - per core, non-self edges are sorted by (dest tile, source group) and
   padded into 128-edge chunks; all chunks of a group are fetched with a few
   large InstDMAGatherAnt instructions (GCH chunks each, single_packet=False,
   one SWDGE queue per group) instead of one indirect DMA per chunk -- the
   per-instruction ~1us Pool descriptor-emission fixed cost was the original
   bottleneck, and per-queue descriptor drain the next one.
 - dma_gather indices are int16, so sources are split into 4 groups of 25088
   rows.  The group of node v is (2*(half of v's local index) + v_core//4),
   which makes each group exactly one contiguous half of one of the two
   half-AllGather output buffers -- the layer-1 AllGather is split into two
   collectives so the first overlaps layer-1 tail compute and the second
   overlaps layer-2's gathers of the first half.  x is pre-permuted on the
   host into the same grouped order so both layers share one index table.
 - self loops never enter the gather path: layer 1 reads their rows with one
   dense DMA; layer 2 reads the SBUF-resident layer-1 outputs directly.
 - per chunk: build S[e, d] = (colw[e]==d) * norm[e] on DVE, accumulate
   aggT[F x 128d] += X_g^T @ S on PE (PSUM, f32).  Per tile: out[d, Fout] =
   aggT^T @ W + b via PE, relu on ACT, store.  All operands bf16.
"""

import numpy as np

N_NODES = 100000
N_EDGES = 640000
IN_CH = 128
OUT_CH = 64
HID = 128
NCORES = 8
NPC = N_NODES // NCORES          # 12500 nodes per core
P = 128
TILES = (NPC + P - 1) // P       # 98 dest tiles per core
PADN = TILES * P                 # 12544 padded rows per core slice
HTILES = (TILES + 1) // 2        # 49 tiles per collective half
HROWS = HTILES * P               # 6272 rows per half per core
NSEG = 4
GRP = 4 * HROWS                  # 25088 rows per gather group (int16-safe)
GCH = 16                         # chunks (x128 rows) per dma_gather
SCRATCH = 36864                  # SWDGE ring: 36864/16 = 2304 descriptors
DT = "bf16"

_CACHE = {}


def _group_of(v):
    """Gather group + row-within-group of source node v (vectorized)."""
    c = v // NPC
    l = v - c * NPC
    lh = l // HROWS
    g = 2 * lh + c // 4
    j = (c % 4) * HROWS + (l - lh * HROWS)
    return g, j


def _preprocess(edge_index):
    """Sort/partition/pad edges; build gather indices and S-build tables.

    Returns (sched, nseg_chunks, cw, nv, idx, perm) where
      sched: list of (tile, grp) per chunk in matmul order; grp==-1 is the
             per-tile self-loop chunk (always first).
      nseg_chunks[g]: chunks in group g (gather slot order).
      cw/nv: [NCORES, 128, C] f32 -- dest-in-tile / norm per chunk slot.
      idx: [NCORES, 128, cols] int16 packed gather indices (shared by both
             layers), columns ordered group-major.
      perm: x_grouped[perm] scatter map -- x_grouped[g*GRP+j] = x[v].
    """
    row = edge_index[0].astype(np.int64)
    col = edge_index[1].astype(np.int64)

    deg = (np.bincount(col, minlength=N_NODES) + 1.0).astype(np.float64)
    dinv = (1.0 / np.sqrt(deg)).astype(np.float32)
    norm = (dinv[row] * dinv[col]).astype(np.float32)

    core = col // NPC
    local = col - core * NPC
    tile = local // P
    colw = (local - tile * P).astype(np.float32)
    seg, jrow = _group_of(row)

    skey = (core * TILES + tile) * NSEG + seg
    order = np.argsort(skey, kind="stable")
    jrow_s = jrow[order]
    colw_s = colw[order]
    norm_s = norm[order]

    counts = np.bincount(skey, minlength=NCORES * TILES * NSEG).reshape(
        NCORES, TILES, NSEG
    )
    n_ts = ((counts + P - 1) // P).max(axis=0)  # [TILES, NSEG]

    sched = []
    chunk0 = np.zeros((TILES, NSEG), dtype=np.int64)
    selfc = np.zeros(TILES, dtype=np.int64)
    for t in range(TILES):
        selfc[t] = len(sched)
        sched.append((t, -1))
        for s in range(NSEG):
            chunk0[t, s] = len(sched)
            for _ in range(n_ts[t, s]):
                sched.append((t, s))
    C = len(sched)

    pos_in_seg = np.full(C, -1, dtype=np.int64)
    nseg_chunks = [0] * NSEG
    for c, (t, s) in enumerate(sched):
        if s >= 0:
            pos_in_seg[c] = nseg_chunks[s]
            nseg_chunks[s] += 1

    cw = np.zeros((NCORES, P, C), dtype=np.float32)
    nv = np.zeros((NCORES, P, C), dtype=np.float32)
    idx = [np.zeros((NCORES, nseg_chunks[s] * P), dtype=np.int16)
           for s in range(NSEG)]

    for t in range(TILES):
        c = selfc[t]
        nd = min(NPC - t * P, P)
        cw[:, :nd, c] = np.arange(nd, dtype=np.float32)
        for m in range(NCORES):
            v0 = m * NPC + t * P
            nv[m, :nd, c] = dinv[v0:v0 + nd] ** 2

    boundaries = np.searchsorted(
        skey[order], np.arange(NCORES * TILES * NSEG + 1), side="left"
    )
    for m in range(NCORES):
        for t in range(TILES):
            for s in range(NSEG):
                k = (m * TILES + t) * NSEG + s
                b0, b1 = boundaries[k], boundaries[k + 1]
                cnt = b1 - b0
                if cnt == 0:
                    continue
                c0 = chunk0[t, s]
                slot = np.arange(cnt)
                ch = slot // P
                part = slot % P
                cw[m, part, c0 + ch] = colw_s[b0:b1]
                nv[m, part, c0 + ch] = norm_s[b0:b1]
                flat = (pos_in_seg[c0] + ch) * P + part
                idx[s][m, flat] = jrow_s[b0:b1].astype(np.int16)

    def pack(seg_arrays):
        packed = []
        for m in range(NCORES):
            cols = []
            for s in range(NSEG):
                a = seg_arrays[s][m]
                t16 = a.reshape(-1, 16).T
                cols.append(np.tile(t16, (8, 1)))
            packed.append(np.concatenate(cols, axis=1))
        return np.ascontiguousarray(np.stack(packed))

    v = np.arange(N_NODES, dtype=np.int64)
    g_all, j_all = _group_of(v)
    perm = g_all * GRP + j_all  # x_grouped[perm[v]] = x[v]

    return sched, nseg_chunks, cw, nv, pack(idx), perm


def _build_module(sched, nseg_chunks, timing_mode=False, variant="full"):
    import concourse.bass as bass
    import concourse.bacc as bacc
    import concourse.tile as tile
    import concourse.mybir as mybir

    f32 = mybir.dt.float32
    i16 = mybir.dt.int16
    i32 = mybir.dt.int32
    dt = mybir.dt.bfloat16 if DT == "bf16" else f32

    C = len(sched)
    seg_col_off = np.concatenate(
        [[0], np.cumsum([nseg_chunks[s] * P // 16 for s in range(NSEG)])]
    ).astype(np.int64)
    idx_cols = int(seg_col_off[-1])
    n_gath = [(nseg_chunks[s] + GCH - 1) // GCH for s in range(NSEG)]

    ndev = 1 if timing_mode else NCORES
    nc = bacc.Bacc(
        "TRN2",
        target_bir_lowering=False,
        debug=False,
        num_devices=ndev,
        num_swdge_queues=4,
        dynamic_dma_scratch_size=SCRATCH,
    )

    xg_d = nc.dram_tensor("xg_d", [NSEG * GRP, IN_CH], dt,
                          kind="ExternalInput")
    xself_d = nc.dram_tensor("xself_d", [TILES, P, IN_CH], dt,
                             kind="ExternalInput")
    idx_d = nc.dram_tensor("idx_d", [P, idx_cols], i16, kind="ExternalInput")
    cw_d = nc.dram_tensor("cw_d", [P, C], f32, kind="ExternalInput")
    nv_d = nc.dram_tensor("nv_d", [P, C], f32, kind="ExternalInput")
    w1_d = nc.dram_tensor("w1_d", [IN_CH, HID], dt, kind="ExternalInput")
    b1_d = nc.dram_tensor("b1_d", [1, HID], dt, kind="ExternalInput")
    w2_d = nc.dram_tensor("w2_d", [HID, OUT_CH], dt, kind="ExternalInput")
    b2_d = nc.dram_tensor("b2_d", [1, OUT_CH], dt, kind="ExternalInput")

    h1_mine = nc.dram_tensor("h1_mine", [TILES, P, HID], dt)
    h1_lo = nc.dram_tensor("h1_lo", [NCORES * HROWS, HID], dt,
                           addr_space="Shared")
    h1_hi = nc.dram_tensor("h1_hi", [NCORES * HROWS, HID], dt,
                           addr_space="Shared")
    out_d = nc.dram_tensor("out_d", [PADN, OUT_CH], f32, kind="ExternalOutput")

    cfg = globals().get("_POOL_CFG") or {}
    with tile.TileContext(nc) as tc:
        with (
            tc.tile_pool(name="const", bufs=1) as cpool,
            tc.tile_pool(name="g0", bufs=cfg.get("SEG_BUFS", 3)) as gp0,
            tc.tile_pool(name="g1", bufs=cfg.get("SEG_BUFS", 3)) as gp1,
            tc.tile_pool(name="g2", bufs=cfg.get("SEG_BUFS", 3)) as gp2,
            tc.tile_pool(name="g3", bufs=cfg.get("SEG_BUFS", 3)) as gp3,
            tc.tile_pool(name="sel", bufs=cfg.get("SEL_BUFS", 8)) as spool,
            tc.tile_pool(name="out", bufs=cfg.get("OUT_BUFS", 6)) as opool,
            tc.tile_pool(name="psA", bufs=cfg.get("PSA_BUFS", 4), space="PSUM") as psA,
            tc.tile_pool(name="psB", bufs=cfg.get("PSB_BUFS", 3), space="PSUM") as psB,
        ):
            gpools = [gp0, gp1, gp2, gp3]

            iota_i = cpool.tile([P, P], i32)
            nc.gpsimd.iota(iota_i[:], pattern=[[1, P]], base=0,
                           channel_multiplier=0)
            iota_f = cpool.tile([P, P], dt)
            nc.vector.tensor_copy(out=iota_f[:], in_=iota_i[:])

            idx_s = cpool.tile([P, idx_cols], i16)
            nc.sync.dma_start(out=idx_s[:], in_=idx_d[:, :])
            cw_s = cpool.tile([P, C], f32)
            nc.sync.dma_start(out=cw_s[:], in_=cw_d[:, :])
            nv_s = cpool.tile([P, C], f32)
            nc.sync.dma_start(out=nv_s[:], in_=nv_d[:, :])

            w1_s = cpool.tile([IN_CH, HID], dt)
            nc.sync.dma_start(out=w1_s[:], in_=w1_d[:, :])
            b1_s = cpool.tile([1, HID], dt)
            nc.sync.dma_start(out=b1_s[:], in_=b1_d[:, :])
            w2_s = cpool.tile([HID, OUT_CH], dt)
            nc.sync.dma_start(out=w2_s[:], in_=w2_d[:, :])
            b2_s = cpool.tile([1, OUT_CH], dt)
            nc.sync.dma_start(out=b2_s[:], in_=b2_d[:, :])
            ones_s = cpool.tile([1, P], dt)
            nc.vector.memset(ones_s[:], 1.0)

            # layer-1 self rows (x slice, padded) and resident layer-1 output
            xself_s = cpool.tile([P, TILES, IN_CH], dt)
            nc.scalar.dma_start(
                out=xself_s[:, :, :],
                in_=xself_d[:, :, :].rearrange("t p f -> p t f"),
            )
            h1self_s = cpool.tile([P, TILES, HID], dt)

            def layer(seg_srcs, self_tiles, w_s, b_s, fout, relu,
                      gathers_only=False, post_tile=None,
                      group_phases=None, mid_hook=None):
                seg_tiles = [[None] * n_gath[s] for s in range(NSEG)]

                def emit_gathers(groups):
                    for g in range(max(n_gath)):
                        for s in groups:
                            if g >= n_gath[s]:
                                continue
                            k = min(GCH, nseg_chunks[s] - g * GCH)
                            xg = gpools[s].tile([P, GCH, IN_CH], dt, tag="xg")
                            nc.gpsimd.dma_gather(
                                xg[:, 0:k, :],
                                seg_srcs[s],
                                idx_s[:, seg_col_off[s] + g * GCH * 8:
                                      seg_col_off[s] + (g * GCH + k) * 8],
                                k * P,
                                k * P,
                                IN_CH,
                                single_packet=False,
                                queue_num=s,
                            )
                            seg_tiles[s][g] = xg

                if group_phases is None:
                    group_phases = [list(range(NSEG))]
                emit_gathers(group_phases[0])
                if mid_hook is not None:
                    mid_hook()
                for ph in group_phases[1:]:
                    emit_gathers(ph)

                if gathers_only:
                    return
                pos = [0] * NSEG
                c = 0
                for t in range(TILES):
                    aggT = psA.tile([P, P], f32, space="PSUM", tag="aggT")
                    nch = 1
                    while c + nch < C and sched[c + nch][0] == t:
                        nch += 1
                    for j in range(nch):
                        tt, s = sched[c + j]
                        if s < 0:
                            lhsT = self_tiles[:, t, :]
                        else:
                            p = pos[s]
                            pos[s] += 1
                            lhsT = seg_tiles[s][p // GCH][:, p % GCH, :]
                        S = spool.tile([P, P], dt, tag="S")
                        nc.vector.tensor_scalar(
                            out=S[:],
                            in0=iota_f[:],
                            scalar1=cw_s[:, c + j:c + j + 1],
                            scalar2=nv_s[:, c + j:c + j + 1],
                            op0=mybir.AluOpType.is_equal,
                            op1=mybir.AluOpType.mult,
                        )
                        nc.tensor.matmul(
                            out=aggT[:],
                            lhsT=lhsT,
                            rhs=S[:],
                            start=(j == 0),
                            stop=(j == nch - 1),
                        )
                    c += nch
                    aggT_s = spool.tile([P, P], dt, tag="aggTs")
                    nc.scalar.copy(out=aggT_s[:], in_=aggT[:])
                    h_ps = psB.tile([P, fout], f32, space="PSUM", tag="h")
                    nc.tensor.matmul(
                        out=h_ps[:], lhsT=ones_s[:], rhs=b_s[:],
                        start=True, stop=False,
                    )
                    nc.tensor.matmul(
                        out=h_ps[:], lhsT=aggT_s[:], rhs=w_s[:],
                        start=False, stop=True,
                    )
                    if relu:
                        nc.scalar.activation(
                            out=h1self_s[:, t, :],
                            in_=h_ps[:],
                            func=mybir.ActivationFunctionType.Relu,
                        )
                        nc.sync.dma_start(out=h1_mine[t, :, :],
                                          in_=h1self_s[:, t, :])
                    else:
                        h_sb = opool.tile([P, fout], f32, tag="ho")
                        nc.vector.tensor_copy(out=h_sb[:], in_=h_ps[:])
                        nc.sync.dma_start(
                            out=out_d[t * P:(t + 1) * P, :], in_=h_sb[:]
                        )
                    if post_tile is not None:
                        post_tile(t)

            def emit_ag(lo):
                nc.gpsimd.collective_compute(
                    "AllGather",
                    mybir.AluOpType.bypass,
                    replica_groups=[list(range(NCORES))],
                    ins=[h1_mine[0:HTILES, :, :].opt() if lo
                         else h1_mine[HTILES:TILES, :, :].opt()],
                    outs=[h1_lo[:, :].opt() if lo else h1_hi[:, :].opt()],
                )

            do_coll = (not timing_mode) and variant in ("full", "coll")

            def post_tile(t):
                if not do_coll:
                    return
                if t == HTILES - 1:
                    emit_ag(lo=True)

            go = variant == "gathers"
            if variant != "coll":
                layer([xg_d[s * GRP:(s + 1) * GRP, :] for s in range(NSEG)],
                      xself_s, w1_s, b1_s, HID, relu=True, gathers_only=go,
                      post_tile=post_tile)
            else:
                t0 = opool.tile([P, HID], dt, tag="ho")
                nc.vector.memset(t0[:], 0.0)
                for t in range(TILES):
                    nc.sync.dma_start(out=h1_mine[t, :, :], in_=t0[:])
                emit_ag(lo=True)
                emit_ag(lo=False)

            if variant not in ("l1", "coll"):
                # lo-group gathers are emitted before AG_hi on the Pool queue
                # so they overlap the second half-collective
                layer([h1_lo[0:GRP, :], h1_lo[GRP:2 * GRP, :],
                       h1_hi[0:GRP, :], h1_hi[GRP:2 * GRP, :]],
                      h1self_s, w2_s, b2_s, OUT_CH, relu=False,
                      gathers_only=go,
                      group_phases=[[0, 1], [2, 3]],
                      mid_hook=(lambda: emit_ag(lo=False)) if do_coll
                      else None)

    nc.compile()
    return nc


def _np_dt():
    if DT == "bf16":
        import ml_dtypes

        return np.dtype(ml_dtypes.bfloat16)
    return np.dtype(np.float32)


def prepare(x, edge_index, W1, b1, W2, b2):
    """Compile (cached) and build per-core input maps."""
    edge_index = np.asarray(edge_index)
    key = hash(edge_index.tobytes())
    if key not in _CACHE:
        sched, nseg_chunks, cw, nv, idx, perm = _preprocess(edge_index)
        nc = _build_module(sched, nseg_chunks)
        _CACHE.clear()
        _CACHE[key] = (nc, sched, nseg_chunks, cw, nv, idx, perm)
    nc, sched, nseg_chunks, cw, nv, idx, perm = _CACHE[key]

    dt = _np_dt()
    x = np.asarray(x, dtype=np.float32).astype(dt)
    xg = np.zeros((NSEG * GRP, IN_CH), dtype=dt)
    xg[perm] = x
    xself = np.zeros((NCORES, TILES, P, IN_CH), dtype=dt)
    for m in range(NCORES):
        xself[m].reshape(PADN, IN_CH)[:NPC] = x[m * NPC:(m + 1) * NPC]
    W1c = np.ascontiguousarray(np.asarray(W1, dtype=np.float32).astype(dt))
    b1c = np.asarray(b1, dtype=np.float32).astype(dt).reshape(1, HID)
    W2c = np.ascontiguousarray(np.asarray(W2, dtype=np.float32).astype(dt))
    b2c = np.asarray(b2, dtype=np.float32).astype(dt).reshape(1, OUT_CH)

    in_maps = [
        {
            "xg_d": xg,
            "xself_d": xself[m],
            "idx_d": idx[m],
            "cw_d": np.ascontiguousarray(cw[m]),
            "nv_d": np.ascontiguousarray(nv[m]),
            "w1_d": W1c,
            "b1_d": b1c,
            "w2_d": W2c,
            "b2_d": b2c,
        }
        for m in range(NCORES)
    ]
    return nc, in_maps


def kernel(x, edge_index, W1, b1, W2, b2):
    from concourse import bass_utils

    nc, in_maps = prepare(x, edge_index, W1, b1, W2, b2)

    # the axon/PJRT execute path occasionally hits a transient
    # device-unrecoverable error; retry a couple of times
    last_err = None
    for _attempt in range(3):
        try:
            res = bass_utils.run_bass_kernel_spmd(
                nc, in_maps, core_ids=list(range(NCORES))
            )
            break
        except Exception as e:  # noqa: BLE001
            last_err = e
            import time as _time

            _time.sleep(5.0)
    else:
        raise last_err
    out = np.concatenate(
        [res.results[m]["out_d"][:NPC] for m in range(NCORES)], axis=0
    )
    return out.astype(np.float32)


# revision 22
# speedup vs baseline: 1.0323x; 1.0323x over previous
"""GCN encoder (2-layer GCNConv) Trainium2 Bass kernel, 8-core SPMD.

out = A_hat @ relu(A_hat @ x @ W1 + b1) @ W2 + b2,  A_hat = D^-1/2 (A+I) D^-1/2

Strategy (1D graph partition by destination node):
 - nodes split into 8 contiguous ranges of 12500; each core owns its range's
   aggregations for both layers.
 - per core, non-self edges are sorted by (dest tile, source group) and
   padded into 128-edge chunks; all chunks of a group are fetched with a few
   large InstDMAGatherAnt instructions (GCH chunks each, single_packet=False,
   one SWDGE queue per group) instead of one indirect DMA per chunk -- the
   per-instruction ~1us Pool descriptor-emission fixed cost was the original
   bottleneck, and per-queue descriptor drain the next one.
 - dma_gather indices are int16, so sources are split into 4 groups of 25088
   rows.  The group of node v is (2*(half of v's local index) + v_core//4),
   which makes each group exactly one contiguous half of one of the two
   half-AllGather output buffers -- the layer-1 AllGather is split into two
   collectives so the first overlaps layer-1 tail compute and the second
   overlaps layer-2's gathers of the first half.  x is pre-permuted on the
   host into the same grouped order so both layers share one index table.
 - self loops never enter the gather path: layer 1 reads their rows with one
   dense DMA; layer 2 reads the SBUF-resident layer-1 outputs directly.
 - per chunk: build S[e, d] = (colw[e]==d) * norm[e] on DVE, accumulate
   aggT[F x 128d] += X_g^T @ S on PE (PSUM, f32).  Per tile: out[d, Fout] =
   aggT^T @ W + b via PE, relu on ACT, store.  All operands bf16.
"""

import numpy as np

N_NODES = 100000
N_EDGES = 640000
IN_CH = 128
OUT_CH = 64
HID = 128
NCORES = 8
NPC = N_NODES // NCORES          # 12500 nodes per core
P = 128
TILES = (NPC + P - 1) // P       # 98 dest tiles per core
PADN = TILES * P                 # 12544 padded rows per core slice
HTILES = (TILES + 1) // 2        # 49 tiles per collective half
HROWS = HTILES * P               # 6272 rows per half per core
NSEG = 4
GRP = 4 * HROWS                  # 25088 rows per gather group (int16-safe)
GCH = 16                         # chunks (x128 rows) per dma_gather
SCRATCH = 36864                  # SWDGE ring: 36864/16 = 2304 descriptors
DT = "bf16"

_CACHE = {}


def _group_of(v):
    """Gather group + row-within-group of source node v (vectorized)."""
    c = v // NPC
    l = v - c * NPC
    lh = l // HROWS
    g = 2 * lh + c // 4
    j = (c % 4) * HROWS + (l - lh * HROWS)
    return g, j


def _preprocess(edge_index):
    """Sort/partition/pad edges; build gather indices and S-build tables.

    Returns (sched, nseg_chunks, cw, nv, idx, perm) where
      sched: list of (tile, grp) per chunk in matmul order; grp==-1 is the
             per-tile self-loop chunk (always first).
      nseg_chunks[g]: chunks in group g (gather slot order).
      cw/nv: [NCORES, 128, C] f32 -- dest-in-tile / norm per chunk slot.
      idx: [NCORES, 128, cols] int16 packed gather indices (shared by both
             layers), columns ordered group-major.
      perm: x_grouped[perm] scatter map -- x_grouped[g*GRP+j] = x[v].
    """
    row = edge_index[0].astype(np.int64)
    col = edge_index[1].astype(np.int64)

    deg = (np.bincount(col, minlength=N_NODES) + 1.0).astype(np.float64)
    dinv = (1.0 / np.sqrt(deg)).astype(np.float32)
    norm = (dinv[row] * dinv[col]).astype(np.float32)

    core = col // NPC
    local = col - core * NPC
    tile = local // P
    colw = (local - tile * P).astype(np.float32)
    seg, jrow = _group_of(row)

    skey = (core * TILES + tile) * NSEG + seg
    order = np.argsort(skey, kind="stable")
    jrow_s = jrow[order]
    colw_s = colw[order]
    norm_s = norm[order]

    counts = np.bincount(skey, minlength=NCORES * TILES * NSEG).reshape(
        NCORES, TILES, NSEG
    )
    n_ts = ((counts + P - 1) // P).max(axis=0)  # [TILES, NSEG]

    sched = []
    chunk0 = np.zeros((TILES, NSEG), dtype=np.int64)
    selfc = np.zeros(TILES, dtype=np.int64)
    for t in range(TILES):
        selfc[t] = len(sched)
        sched.append((t, -1))
        for s in range(NSEG):
            chunk0[t, s] = len(sched)
            for _ in range(n_ts[t, s]):
                sched.append((t, s))
    C = len(sched)

    pos_in_seg = np.full(C, -1, dtype=np.int64)
    nseg_chunks = [0] * NSEG
    for c, (t, s) in enumerate(sched):
        if s >= 0:
            pos_in_seg[c] = nseg_chunks[s]
            nseg_chunks[s] += 1

    cw = np.zeros((NCORES, P, C), dtype=np.float32)
    nv = np.zeros((NCORES, P, C), dtype=np.float32)
    idx = [np.zeros((NCORES, nseg_chunks[s] * P), dtype=np.int16)
           for s in range(NSEG)]

    for t in range(TILES):
        c = selfc[t]
        nd = min(NPC - t * P, P)
        cw[:, :nd, c] = np.arange(nd, dtype=np.float32)
        for m in range(NCORES):
            v0 = m * NPC + t * P
            nv[m, :nd, c] = dinv[v0:v0 + nd] ** 2

    boundaries = np.searchsorted(
        skey[order], np.arange(NCORES * TILES * NSEG + 1), side="left"
    )
    for m in range(NCORES):
        for t in range(TILES):
            for s in range(NSEG):
                k = (m * TILES + t) * NSEG + s
                b0, b1 = boundaries[k], boundaries[k + 1]
                cnt = b1 - b0
                if cnt == 0:
                    continue
                c0 = chunk0[t, s]
                slot = np.arange(cnt)
                ch = slot // P
                part = slot % P
                cw[m, part, c0 + ch] = colw_s[b0:b1]
                nv[m, part, c0 + ch] = norm_s[b0:b1]
                flat = (pos_in_seg[c0] + ch) * P + part
                idx[s][m, flat] = jrow_s[b0:b1].astype(np.int16)

    def pack(seg_arrays):
        packed = []
        for m in range(NCORES):
            cols = []
            for s in range(NSEG):
                a = seg_arrays[s][m]
                t16 = a.reshape(-1, 16).T
                cols.append(np.tile(t16, (8, 1)))
            packed.append(np.concatenate(cols, axis=1))
        return np.ascontiguousarray(np.stack(packed))

    v = np.arange(N_NODES, dtype=np.int64)
    g_all, j_all = _group_of(v)
    perm = g_all * GRP + j_all  # x_grouped[perm[v]] = x[v]

    return sched, nseg_chunks, cw, nv, pack(idx), perm


def _build_module(sched, nseg_chunks, timing_mode=False, variant="full"):
    import concourse.bass as bass
    import concourse.bacc as bacc
    import concourse.tile as tile
    import concourse.mybir as mybir

    f32 = mybir.dt.float32
    i16 = mybir.dt.int16
    i32 = mybir.dt.int32
    dt = mybir.dt.bfloat16 if DT == "bf16" else f32

    C = len(sched)
    seg_col_off = np.concatenate(
        [[0], np.cumsum([nseg_chunks[s] * P // 16 for s in range(NSEG)])]
    ).astype(np.int64)
    idx_cols = int(seg_col_off[-1])
    n_gath = [(nseg_chunks[s] + GCH - 1) // GCH for s in range(NSEG)]

    ndev = 1 if timing_mode else NCORES
    nc = bacc.Bacc(
        "TRN2",
        target_bir_lowering=False,
        debug=False,
        num_devices=ndev,
        num_swdge_queues=4,
        dynamic_dma_scratch_size=SCRATCH,
    )

    xg_d = nc.dram_tensor("xg_d", [NSEG * GRP, IN_CH], dt,
                          kind="ExternalInput")
    xself_d = nc.dram_tensor("xself_d", [TILES, P, IN_CH], dt,
                             kind="ExternalInput")
    idx_d = nc.dram_tensor("idx_d", [P, idx_cols], i16, kind="ExternalInput")
    cw_d = nc.dram_tensor("cw_d", [P, C], f32, kind="ExternalInput")
    nv_d = nc.dram_tensor("nv_d", [P, C], f32, kind="ExternalInput")
    w1_d = nc.dram_tensor("w1_d", [IN_CH, HID], dt, kind="ExternalInput")
    b1_d = nc.dram_tensor("b1_d", [1, HID], dt, kind="ExternalInput")
    w2_d = nc.dram_tensor("w2_d", [HID, OUT_CH], dt, kind="ExternalInput")
    b2_d = nc.dram_tensor("b2_d", [1, OUT_CH], dt, kind="ExternalInput")

    h1_mine = nc.dram_tensor("h1_mine", [TILES, P, HID], dt)
    h1_lo = nc.dram_tensor("h1_lo", [NCORES * HROWS, HID], dt,
                           addr_space="Shared")
    h1_hi = nc.dram_tensor("h1_hi", [NCORES * HROWS, HID], dt,
                           addr_space="Shared")
    out_d = nc.dram_tensor("out_d", [PADN, OUT_CH], f32, kind="ExternalOutput")

    cfg = globals().get("_POOL_CFG") or {}
    with tile.TileContext(nc) as tc:
        with (
            tc.tile_pool(name="const", bufs=1) as cpool,
            tc.tile_pool(name="g0", bufs=cfg.get("SEG_BUFS", 3)) as gp0,
            tc.tile_pool(name="g1", bufs=cfg.get("SEG_BUFS", 3)) as gp1,
            tc.tile_pool(name="g2", bufs=cfg.get("SEG_BUFS", 3)) as gp2,
            tc.tile_pool(name="g3", bufs=cfg.get("SEG_BUFS", 3)) as gp3,
            tc.tile_pool(name="sel", bufs=cfg.get("SEL_BUFS", 8)) as spool,
            tc.tile_pool(name="out", bufs=cfg.get("OUT_BUFS", 6)) as opool,
            tc.tile_pool(name="psA", bufs=cfg.get("PSA_BUFS", 4), space="PSUM") as psA,
            tc.tile_pool(name="psB", bufs=cfg.get("PSB_BUFS", 3), space="PSUM") as psB,
        ):
            gpools = [gp0, gp1, gp2, gp3]

            iota_i = cpool.tile([P, P], i32)
            nc.gpsimd.iota(iota_i[:], pattern=[[1, P]], base=0,
                           channel_multiplier=0)
            iota_f = cpool.tile([P, P], dt)
            nc.vector.tensor_copy(out=iota_f[:], in_=iota_i[:])

            idx_s = cpool.tile([P, idx_cols], i16)
            nc.sync.dma_start(out=idx_s[:], in_=idx_d[:, :])
            cw_s = cpool.tile([P, C], f32)
            nc.sync.dma_start(out=cw_s[:], in_=cw_d[:, :])
            nv_s = cpool.tile([P, C], f32)
            nc.sync.dma_start(out=nv_s[:], in_=nv_d[:, :])

            w1_s = cpool.tile([IN_CH, HID], dt)
            nc.sync.dma_start(out=w1_s[:], in_=w1_d[:, :])
            b1_s = cpool.tile([1, HID], dt)
            nc.sync.dma_start(out=b1_s[:], in_=b1_d[:, :])
            w2_s = cpool.tile([HID, OUT_CH], dt)
            nc.sync.dma_start(out=w2_s[:], in_=w2_d[:, :])
            b2_s = cpool.tile([1, OUT_CH], dt)
            nc.sync.dma_start(out=b2_s[:], in_=b2_d[:, :])
            ones_s = cpool.tile([1, P], dt)
            nc.vector.memset(ones_s[:], 1.0)

            # layer-1 self rows (x slice, padded) and resident layer-1 output
            xself_s = cpool.tile([P, TILES, IN_CH], dt)
            nc.scalar.dma_start(
                out=xself_s[:, :, :],
                in_=xself_d[:, :, :].rearrange("t p f -> p t f"),
            )
            h1self_s = cpool.tile([P, TILES, HID], dt)

            # SWDGE queue must follow the tile framework's DMASW lane
            # round-robin (lane = emission_index % 8, so queue must be
            # emission_index % 4) or sem lanes get cross-queue updates
            swdge_ctr = [0]

            def layer(seg_srcs, self_tiles, w_s, b_s, fout, relu,
                      gathers_only=False, post_tile=None,
                      group_phases=None, mid_hook=None):
                seg_tiles = [[None] * n_gath[s] for s in range(NSEG)]

                def emit_gathers(groups):
                    for g in range(max(n_gath)):
                        for s in groups:
                            if g >= n_gath[s]:
                                continue
                            k = min(GCH, nseg_chunks[s] - g * GCH)
                            xg = gpools[s].tile([P, GCH, IN_CH], dt, tag="xg")
                            nc.gpsimd.dma_gather(
                                xg[:, 0:k, :],
                                seg_srcs[s],
                                idx_s[:, seg_col_off[s] + g * GCH * 8:
                                      seg_col_off[s] + (g * GCH + k) * 8],
                                k * P,
                                k * P,
                                IN_CH,
                                single_packet=False,
                                queue_num=swdge_ctr[0] % 4,
                            )
                            swdge_ctr[0] += 1
                            seg_tiles[s][g] = xg

                if group_phases is None:
                    group_phases = [list(range(NSEG))]
                emit_gathers(group_phases[0])
                if mid_hook is not None:
                    mid_hook()
                for ph in group_phases[1:]:
                    emit_gathers(ph)

                if gathers_only:
                    return
                pos = [0] * NSEG
                c = 0
                for t in range(TILES):
                    aggT = psA.tile([P, P], f32, space="PSUM", tag="aggT")
                    nch = 1
                    while c + nch < C and sched[c + nch][0] == t:
                        nch += 1
                    for j in range(nch):
                        tt, s = sched[c + j]
                        if s < 0:
                            lhsT = self_tiles[:, t, :]
                        else:
                            p = pos[s]
                            pos[s] += 1
                            lhsT = seg_tiles[s][p // GCH][:, p % GCH, :]
                        S = spool.tile([P, P], dt, tag="S")
                        nc.vector.tensor_scalar(
                            out=S[:],
                            in0=iota_f[:],
                            scalar1=cw_s[:, c + j:c + j + 1],
                            scalar2=nv_s[:, c + j:c + j + 1],
                            op0=mybir.AluOpType.is_equal,
                            op1=mybir.AluOpType.mult,
                        )
                        nc.tensor.matmul(
                            out=aggT[:],
                            lhsT=lhsT,
                            rhs=S[:],
                            start=(j == 0),
                            stop=(j == nch - 1),
                        )
                    c += nch
                    aggT_s = spool.tile([P, P], dt, tag="aggTs")
                    nc.scalar.copy(out=aggT_s[:], in_=aggT[:])
                    h_ps = psB.tile([P, fout], f32, space="PSUM", tag="h")
                    nc.tensor.matmul(
                        out=h_ps[:], lhsT=ones_s[:], rhs=b_s[:],
                        start=True, stop=False,
                    )
                    nc.tensor.matmul(
                        out=h_ps[:], lhsT=aggT_s[:], rhs=w_s[:],
                        start=False, stop=True,
                    )
                    if relu:
                        nc.scalar.activation(
                            out=h1self_s[:, t, :],
                            in_=h_ps[:],
                            func=mybir.ActivationFunctionType.Relu,
                        )
                        nc.sync.dma_start(out=h1_mine[t, :, :],
                                          in_=h1self_s[:, t, :])
                    else:
                        h_sb = opool.tile([P, fout], f32, tag="ho")
                        nc.vector.tensor_copy(out=h_sb[:], in_=h_ps[:])
                        nc.sync.dma_start(
                            out=out_d[t * P:(t + 1) * P, :], in_=h_sb[:]
                        )
                    if post_tile is not None:
                        post_tile(t)

            def emit_ag(lo):
                nc.gpsimd.collective_compute(
                    "AllGather",
                    mybir.AluOpType.bypass,
                    replica_groups=[list(range(NCORES))],
                    ins=[h1_mine[0:HTILES, :, :].opt() if lo
                         else h1_mine[HTILES:TILES, :, :].opt()],
                    outs=[h1_lo[:, :].opt() if lo else h1_hi[:, :].opt()],
                )

            do_coll = (not timing_mode) and variant in ("full", "coll")

            def post_tile(t):
                if not do_coll:
                    return
                if t == HTILES - 1:
                    emit_ag(lo=True)

            go = variant == "gathers"
            if variant != "coll":
                layer([xg_d[s * GRP:(s + 1) * GRP, :] for s in range(NSEG)],
                      xself_s, w1_s, b1_s, HID, relu=True, gathers_only=go,
                      post_tile=post_tile)
            else:
                t0 = opool.tile([P, HID], dt, tag="ho")
                nc.vector.memset(t0[:], 0.0)
                for t in range(TILES):
                    nc.sync.dma_start(out=h1_mine[t, :, :], in_=t0[:])
                emit_ag(lo=True)
                emit_ag(lo=False)

            if variant not in ("l1", "coll"):
                # lo-group gathers are emitted before AG_hi on the Pool queue
                # so they overlap the second half-collective
                layer([h1_lo[0:GRP, :], h1_lo[GRP:2 * GRP, :],
                       h1_hi[0:GRP, :], h1_hi[GRP:2 * GRP, :]],
                      h1self_s, w2_s, b2_s, OUT_CH, relu=False,
                      gathers_only=go,
                      group_phases=[[0, 1], [2, 3]],
                      mid_hook=(lambda: emit_ag(lo=False)) if do_coll
                      else None)

    nc.compile()
    return nc


def _np_dt():
    if DT == "bf16":
        import ml_dtypes

        return np.dtype(ml_dtypes.bfloat16)
    return np.dtype(np.float32)


def prepare(x, edge_index, W1, b1, W2, b2):
    """Compile (cached) and build per-core input maps."""
    edge_index = np.asarray(edge_index)
    key = hash(edge_index.tobytes())
    if key not in _CACHE:
        sched, nseg_chunks, cw, nv, idx, perm = _preprocess(edge_index)
        nc = _build_module(sched, nseg_chunks)
        _CACHE.clear()
        _CACHE[key] = (nc, sched, nseg_chunks, cw, nv, idx, perm)
    nc, sched, nseg_chunks, cw, nv, idx, perm = _CACHE[key]

    dt = _np_dt()
    x = np.asarray(x, dtype=np.float32).astype(dt)
    xg = np.zeros((NSEG * GRP, IN_CH), dtype=dt)
    xg[perm] = x
    xself = np.zeros((NCORES, TILES, P, IN_CH), dtype=dt)
    for m in range(NCORES):
        xself[m].reshape(PADN, IN_CH)[:NPC] = x[m * NPC:(m + 1) * NPC]
    W1c = np.ascontiguousarray(np.asarray(W1, dtype=np.float32).astype(dt))
    b1c = np.asarray(b1, dtype=np.float32).astype(dt).reshape(1, HID)
    W2c = np.ascontiguousarray(np.asarray(W2, dtype=np.float32).astype(dt))
    b2c = np.asarray(b2, dtype=np.float32).astype(dt).reshape(1, OUT_CH)

    in_maps = [
        {
            "xg_d": xg,
            "xself_d": xself[m],
            "idx_d": idx[m],
            "cw_d": np.ascontiguousarray(cw[m]),
            "nv_d": np.ascontiguousarray(nv[m]),
            "w1_d": W1c,
            "b1_d": b1c,
            "w2_d": W2c,
            "b2_d": b2c,
        }
        for m in range(NCORES)
    ]
    return nc, in_maps


def kernel(x, edge_index, W1, b1, W2, b2):
    from concourse import bass_utils

    nc, in_maps = prepare(x, edge_index, W1, b1, W2, b2)

    # the axon/PJRT execute path occasionally hits a transient
    # device-unrecoverable error; retry a couple of times
    last_err = None
    for _attempt in range(3):
        try:
            res = bass_utils.run_bass_kernel_spmd(
                nc, in_maps, core_ids=list(range(NCORES))
            )
            break
        except Exception as e:  # noqa: BLE001
            last_err = e
            import time as _time

            _time.sleep(5.0)
    else:
        raise last_err
    out = np.concatenate(
        [res.results[m]["out_d"][:NPC] for m in range(NCORES)], axis=0
    )
    return out.astype(np.float32)


# revision 24
# speedup vs baseline: 1.0932x; 1.0590x over previous
"""GCN encoder (2-layer GCNConv) Trainium2 Bass kernel, 8-core SPMD.

out = A_hat @ relu(A_hat @ x @ W1 + b1) @ W2 + b2,  A_hat = D^-1/2 (A+I) D^-1/2

Strategy (1D graph partition by destination node):
 - nodes split into 8 contiguous ranges of 12500; each core owns its range's
   aggregations for both layers.
 - per core, non-self edges are sorted by (dest tile, source group) and
   padded into 128-edge chunks; all chunks of a group are fetched with a few
   large InstDMAGatherAnt instructions (GCH chunks each, single_packet=False,
   one SWDGE queue per group) instead of one indirect DMA per chunk -- the
   per-instruction ~1us Pool descriptor-emission fixed cost was the original
   bottleneck, and per-queue descriptor drain the next one.
 - dma_gather indices are int16, so sources are split into 4 groups of 25088
   rows.  The group of node v is (2*(half of v's local index) + v_core//4),
   which makes each group exactly one contiguous half of one of the two
   half-AllGather output buffers -- the layer-1 AllGather is split into two
   collectives so the first overlaps layer-1 tail compute and the second
   overlaps layer-2's gathers of the first half.  x is pre-permuted on the
   host into the same grouped order so both layers share one index table.
 - self loops never enter the gather path: layer 1 reads their rows with one
   dense DMA; layer 2 reads the SBUF-resident layer-1 outputs directly.
 - per chunk: build S[e, d] = (colw[e]==d) * norm[e] on DVE, accumulate
   aggT[F x 128d] += X_g^T @ S on PE (PSUM, f32).  Per tile: out[d, Fout] =
   aggT^T @ W + b via PE, relu on ACT, store.  All operands bf16.
"""

import numpy as np

N_NODES = 100000
N_EDGES = 640000
IN_CH = 128
OUT_CH = 64
HID = 128
NCORES = 8
NPC = N_NODES // NCORES          # 12500 nodes per core
P = 128
TILES = (NPC + P - 1) // P       # 98 dest tiles per core
PADN = TILES * P                 # 12544 padded rows per core slice
HTILES = (TILES + 1) // 2        # 49 tiles per collective half
HROWS = HTILES * P               # 6272 rows per half per core
NSEG = 4
GRP = 4 * HROWS                  # 25088 rows per gather group (int16-safe)
GCH = 16                         # chunks (x128 rows) per dma_gather
SCRATCH = 36864                  # SWDGE ring: 36864/16 = 2304 descriptors
DT = "bf16"

_CACHE = {}


def _group_of(v):
    """Gather group + row-within-group of source node v (vectorized)."""
    c = v // NPC
    l = v - c * NPC
    lh = l // HROWS
    g = 2 * lh + c // 4
    j = (c % 4) * HROWS + (l - lh * HROWS)
    return g, j


def _preprocess(edge_index):
    """Sort/partition/pad edges; build gather indices and S-build tables.

    Returns (sched, nseg_chunks, cw, nv, idx, perm) where
      sched: list of (tile, grp) per chunk in matmul order; grp==-1 is the
             per-tile self-loop chunk (always first).
      nseg_chunks[g]: chunks in group g (gather slot order).
      cw/nv: [NCORES, 128, C] f32 -- dest-in-tile / norm per chunk slot.
      idx: [NCORES, 128, cols] int16 packed gather indices (shared by both
             layers), columns ordered group-major.
      perm: x_grouped[perm] scatter map -- x_grouped[g*GRP+j] = x[v].
    """
    row = edge_index[0].astype(np.int64)
    col = edge_index[1].astype(np.int64)

    deg = (np.bincount(col, minlength=N_NODES) + 1.0).astype(np.float64)
    dinv = (1.0 / np.sqrt(deg)).astype(np.float32)
    norm = (dinv[row] * dinv[col]).astype(np.float32)

    core = col // NPC
    local = col - core * NPC
    tile = local // P
    colw = (local - tile * P).astype(np.float32)
    seg, jrow = _group_of(row)

    skey = (core * TILES + tile) * NSEG + seg
    order = np.argsort(skey, kind="stable")
    jrow_s = jrow[order]
    colw_s = colw[order]
    norm_s = norm[order]

    counts = np.bincount(skey, minlength=NCORES * TILES * NSEG).reshape(
        NCORES, TILES, NSEG
    )
    n_ts = ((counts + P - 1) // P).max(axis=0)  # [TILES, NSEG]

    sched = []
    chunk0 = np.zeros((TILES, NSEG), dtype=np.int64)
    selfc = np.zeros(TILES, dtype=np.int64)
    for t in range(TILES):
        selfc[t] = len(sched)
        sched.append((t, -1))
        for s in range(NSEG):
            chunk0[t, s] = len(sched)
            for _ in range(n_ts[t, s]):
                sched.append((t, s))
    C = len(sched)

    pos_in_seg = np.full(C, -1, dtype=np.int64)
    nseg_chunks = [0] * NSEG
    for c, (t, s) in enumerate(sched):
        if s >= 0:
            pos_in_seg[c] = nseg_chunks[s]
            nseg_chunks[s] += 1

    cw = np.zeros((NCORES, P, C), dtype=np.float32)
    nv = np.zeros((NCORES, P, C), dtype=np.float32)
    idx = [np.zeros((NCORES, nseg_chunks[s] * P), dtype=np.int16)
           for s in range(NSEG)]

    for t in range(TILES):
        c = selfc[t]
        nd = min(NPC - t * P, P)
        cw[:, :nd, c] = np.arange(nd, dtype=np.float32)
        for m in range(NCORES):
            v0 = m * NPC + t * P
            nv[m, :nd, c] = dinv[v0:v0 + nd] ** 2

    boundaries = np.searchsorted(
        skey[order], np.arange(NCORES * TILES * NSEG + 1), side="left"
    )
    for m in range(NCORES):
        for t in range(TILES):
            for s in range(NSEG):
                k = (m * TILES + t) * NSEG + s
                b0, b1 = boundaries[k], boundaries[k + 1]
                cnt = b1 - b0
                if cnt == 0:
                    continue
                c0 = chunk0[t, s]
                slot = np.arange(cnt)
                ch = slot // P
                part = slot % P
                cw[m, part, c0 + ch] = colw_s[b0:b1]
                nv[m, part, c0 + ch] = norm_s[b0:b1]
                flat = (pos_in_seg[c0] + ch) * P + part
                idx[s][m, flat] = jrow_s[b0:b1].astype(np.int16)

    def pack(seg_arrays):
        packed = []
        for m in range(NCORES):
            cols = []
            for s in range(NSEG):
                a = seg_arrays[s][m]
                t16 = a.reshape(-1, 16).T
                cols.append(np.tile(t16, (8, 1)))
            packed.append(np.concatenate(cols, axis=1))
        return np.ascontiguousarray(np.stack(packed))

    v = np.arange(N_NODES, dtype=np.int64)
    g_all, j_all = _group_of(v)
    perm = g_all * GRP + j_all  # x_grouped[perm[v]] = x[v]

    return sched, nseg_chunks, cw, nv, pack(idx), perm


def _build_module(sched, nseg_chunks, timing_mode=False, variant="full"):
    import concourse.bass as bass
    import concourse.bacc as bacc
    import concourse.tile as tile
    import concourse.mybir as mybir

    f32 = mybir.dt.float32
    i16 = mybir.dt.int16
    i32 = mybir.dt.int32
    dt = mybir.dt.bfloat16 if DT == "bf16" else f32

    C = len(sched)
    seg_col_off = np.concatenate(
        [[0], np.cumsum([nseg_chunks[s] * P // 16 for s in range(NSEG)])]
    ).astype(np.int64)
    idx_cols = int(seg_col_off[-1])
    n_gath = [(nseg_chunks[s] + GCH - 1) // GCH for s in range(NSEG)]

    ndev = 1 if timing_mode else NCORES
    nc = bacc.Bacc(
        "TRN2",
        target_bir_lowering=False,
        debug=False,
        num_devices=ndev,
        num_swdge_queues=4,
        dynamic_dma_scratch_size=SCRATCH,
    )

    xg_d = nc.dram_tensor("xg_d", [NSEG * GRP, IN_CH], dt,
                          kind="ExternalInput")
    xself_d = nc.dram_tensor("xself_d", [TILES, P, IN_CH], dt,
                             kind="ExternalInput")
    idx_d = nc.dram_tensor("idx_d", [P, idx_cols], i16, kind="ExternalInput")
    cw_d = nc.dram_tensor("cw_d", [P, C], f32, kind="ExternalInput")
    nv_d = nc.dram_tensor("nv_d", [P, C], f32, kind="ExternalInput")
    w1_d = nc.dram_tensor("w1_d", [IN_CH, HID], dt, kind="ExternalInput")
    b1_d = nc.dram_tensor("b1_d", [1, HID], dt, kind="ExternalInput")
    w2_d = nc.dram_tensor("w2_d", [HID, OUT_CH], dt, kind="ExternalInput")
    b2_d = nc.dram_tensor("b2_d", [1, OUT_CH], dt, kind="ExternalInput")

    h1_mine = nc.dram_tensor("h1_mine", [TILES, P, HID], dt)
    h1_lo = nc.dram_tensor("h1_lo", [NCORES * HROWS, HID], dt,
                           addr_space="Shared")
    h1_hi = nc.dram_tensor("h1_hi", [NCORES * HROWS, HID], dt,
                           addr_space="Shared")
    out_d = nc.dram_tensor("out_d", [PADN, OUT_CH], f32, kind="ExternalOutput")

    cfg = globals().get("_POOL_CFG") or {}
    with tile.TileContext(nc) as tc:
        with (
            tc.tile_pool(name="const", bufs=1) as cpool,
            tc.tile_pool(name="g0", bufs=cfg.get("SEG_BUFS", 3)) as gp0,
            tc.tile_pool(name="g1", bufs=cfg.get("SEG_BUFS", 3)) as gp1,
            tc.tile_pool(name="g2", bufs=cfg.get("SEG_BUFS", 3)) as gp2,
            tc.tile_pool(name="g3", bufs=cfg.get("SEG_BUFS", 3)) as gp3,
            tc.tile_pool(name="sel", bufs=cfg.get("SEL_BUFS", 8)) as spool,
            tc.tile_pool(name="out", bufs=cfg.get("OUT_BUFS", 6)) as opool,
            tc.tile_pool(name="psA", bufs=cfg.get("PSA_BUFS", 4), space="PSUM") as psA,
            tc.tile_pool(name="psB", bufs=cfg.get("PSB_BUFS", 3), space="PSUM") as psB,
        ):
            gpools = [gp0, gp1, gp2, gp3]

            iota_i = cpool.tile([P, P], i32)
            nc.gpsimd.iota(iota_i[:], pattern=[[1, P]], base=0,
                           channel_multiplier=0)
            iota_f = cpool.tile([P, P], dt)
            nc.vector.tensor_copy(out=iota_f[:], in_=iota_i[:])

            idx_s = cpool.tile([P, idx_cols], i16)
            nc.sync.dma_start(out=idx_s[:], in_=idx_d[:, :])
            cw_s = cpool.tile([P, C], f32)
            nc.sync.dma_start(out=cw_s[:], in_=cw_d[:, :])
            nv_s = cpool.tile([P, C], f32)
            nc.sync.dma_start(out=nv_s[:], in_=nv_d[:, :])

            w1_s = cpool.tile([IN_CH, HID], dt)
            nc.sync.dma_start(out=w1_s[:], in_=w1_d[:, :])
            b1_s = cpool.tile([1, HID], dt)
            nc.sync.dma_start(out=b1_s[:], in_=b1_d[:, :])
            w2_s = cpool.tile([HID, OUT_CH], dt)
            nc.sync.dma_start(out=w2_s[:], in_=w2_d[:, :])
            b2_s = cpool.tile([1, OUT_CH], dt)
            nc.sync.dma_start(out=b2_s[:], in_=b2_d[:, :])
            ones_s = cpool.tile([1, P], dt)
            nc.vector.memset(ones_s[:], 1.0)

            # layer-1 self rows (x slice, padded) and resident layer-1 output
            xself_s = cpool.tile([P, TILES, IN_CH], dt)
            nc.scalar.dma_start(
                out=xself_s[:, :, :],
                in_=xself_d[:, :, :].rearrange("t p f -> p t f"),
            )
            h1self_s = cpool.tile([P, TILES, HID], dt)

            # SWDGE queue must follow the tile framework's DMASW lane
            # round-robin (lane = emission_index % 8, so queue must be
            # emission_index % 4) or sem lanes get cross-queue updates
            swdge_ctr = [0]

            def layer(seg_srcs, self_tiles, w_s, b_s, fout, relu,
                      gathers_only=False, post_tile=None,
                      group_phases=None, mid_hook=None):
                seg_tiles = [[None] * n_gath[s] for s in range(NSEG)]

                def emit_gathers(groups):
                    for g in range(max(n_gath)):
                        for s in groups:
                            if g >= n_gath[s]:
                                continue
                            k = min(GCH, nseg_chunks[s] - g * GCH)
                            xg = gpools[s].tile([P, GCH, IN_CH], dt, tag="xg")
                            nc.gpsimd.dma_gather(
                                xg[:, 0:k, :],
                                seg_srcs[s],
                                idx_s[:, seg_col_off[s] + g * GCH * 8:
                                      seg_col_off[s] + (g * GCH + k) * 8],
                                k * P,
                                k * P,
                                IN_CH,
                                single_packet=False,
                                queue_num=swdge_ctr[0] % 4,
                            )
                            swdge_ctr[0] += 1
                            seg_tiles[s][g] = xg

                if group_phases is None:
                    group_phases = [list(range(NSEG))]
                emit_gathers(group_phases[0])
                if mid_hook is not None:
                    mid_hook()
                for ph in group_phases[1:]:
                    emit_gathers(ph)

                if gathers_only:
                    return
                pos = [0] * NSEG
                c = 0
                for t in range(TILES):
                    aggT = psA.tile([P, P], f32, space="PSUM", tag="aggT")
                    nch = 1
                    while c + nch < C and sched[c + nch][0] == t:
                        nch += 1
                    for j in range(nch):
                        tt, s = sched[c + j]
                        if s < 0:
                            lhsT = self_tiles[:, t, :]
                        else:
                            p = pos[s]
                            pos[s] += 1
                            lhsT = seg_tiles[s][p // GCH][:, p % GCH, :]
                        S = spool.tile([P, P], dt, tag="S")
                        nc.vector.tensor_scalar(
                            out=S[:],
                            in0=iota_f[:],
                            scalar1=cw_s[:, c + j:c + j + 1],
                            scalar2=nv_s[:, c + j:c + j + 1],
                            op0=mybir.AluOpType.is_equal,
                            op1=mybir.AluOpType.mult,
                        )
                        nc.tensor.matmul(
                            out=aggT[:],
                            lhsT=lhsT,
                            rhs=S[:],
                            start=(j == 0),
                            stop=(j == nch - 1),
                        )
                    c += nch
                    aggT_s = spool.tile([P, P], dt, tag="aggTs")
                    nc.scalar.copy(out=aggT_s[:], in_=aggT[:])
                    h_ps = psB.tile([P, fout], f32, space="PSUM", tag="h")
                    nc.tensor.matmul(
                        out=h_ps[:], lhsT=ones_s[:], rhs=b_s[:],
                        start=True, stop=False,
                    )
                    nc.tensor.matmul(
                        out=h_ps[:], lhsT=aggT_s[:], rhs=w_s[:],
                        start=False, stop=True,
                    )
                    if relu:
                        nc.scalar.activation(
                            out=h1self_s[:, t, :],
                            in_=h_ps[:],
                            func=mybir.ActivationFunctionType.Relu,
                        )
                        nc.sync.dma_start(out=h1_mine[t, :, :],
                                          in_=h1self_s[:, t, :])
                    else:
                        h_sb = opool.tile([P, fout], f32, tag="ho")
                        nc.vector.tensor_copy(out=h_sb[:], in_=h_ps[:])
                        nc.sync.dma_start(
                            out=out_d[t * P:(t + 1) * P, :], in_=h_sb[:]
                        )
                    if post_tile is not None:
                        post_tile(t)

            def emit_ag(lo):
                nc.gpsimd.collective_compute(
                    "AllGather",
                    mybir.AluOpType.bypass,
                    replica_groups=[list(range(NCORES))],
                    ins=[h1_mine[0:HTILES, :, :].opt() if lo
                         else h1_mine[HTILES:TILES, :, :].opt()],
                    outs=[h1_lo[:, :].opt() if lo else h1_hi[:, :].opt()],
                )

            do_coll = (not timing_mode) and variant in ("full", "coll")

            def post_tile(t):
                if not do_coll:
                    return
                if t == HTILES - 1:
                    emit_ag(lo=True)

            go = variant == "gathers"
            if variant != "coll":
                layer([xg_d[s * GRP:(s + 1) * GRP, :] for s in range(NSEG)],
                      xself_s, w1_s, b1_s, HID, relu=True, gathers_only=go,
                      post_tile=post_tile)
            else:
                t0 = opool.tile([P, HID], dt, tag="ho")
                nc.vector.memset(t0[:], 0.0)
                for t in range(TILES):
                    nc.sync.dma_start(out=h1_mine[t, :, :], in_=t0[:])
                emit_ag(lo=True)
                emit_ag(lo=False)

            if do_coll and variant != "coll":
                emit_ag(lo=False)
            if variant not in ("l1", "coll"):
                # lo-group gathers depend only on AG_lo's output, so they
                # overlap AG_hi (collectives complete on their own lane)
                layer([h1_lo[0:GRP, :], h1_lo[GRP:2 * GRP, :],
                       h1_hi[0:GRP, :], h1_hi[GRP:2 * GRP, :]],
                      h1self_s, w2_s, b2_s, OUT_CH, relu=False,
                      gathers_only=go)

    nc.compile()
    return nc


def _np_dt():
    if DT == "bf16":
        import ml_dtypes

        return np.dtype(ml_dtypes.bfloat16)
    return np.dtype(np.float32)


def prepare(x, edge_index, W1, b1, W2, b2):
    """Compile (cached) and build per-core input maps."""
    edge_index = np.asarray(edge_index)
    key = hash(edge_index.tobytes())
    if key not in _CACHE:
        sched, nseg_chunks, cw, nv, idx, perm = _preprocess(edge_index)
        nc = _build_module(sched, nseg_chunks)
        _CACHE.clear()
        _CACHE[key] = (nc, sched, nseg_chunks, cw, nv, idx, perm)
    nc, sched, nseg_chunks, cw, nv, idx, perm = _CACHE[key]

    dt = _np_dt()
    x = np.asarray(x, dtype=np.float32).astype(dt)
    xg = np.zeros((NSEG * GRP, IN_CH), dtype=dt)
    xg[perm] = x
    xself = np.zeros((NCORES, TILES, P, IN_CH), dtype=dt)
    for m in range(NCORES):
        xself[m].reshape(PADN, IN_CH)[:NPC] = x[m * NPC:(m + 1) * NPC]
    W1c = np.ascontiguousarray(np.asarray(W1, dtype=np.float32).astype(dt))
    b1c = np.asarray(b1, dtype=np.float32).astype(dt).reshape(1, HID)
    W2c = np.ascontiguousarray(np.asarray(W2, dtype=np.float32).astype(dt))
    b2c = np.asarray(b2, dtype=np.float32).astype(dt).reshape(1, OUT_CH)

    in_maps = [
        {
            "xg_d": xg,
            "xself_d": xself[m],
            "idx_d": idx[m],
            "cw_d": np.ascontiguousarray(cw[m]),
            "nv_d": np.ascontiguousarray(nv[m]),
            "w1_d": W1c,
            "b1_d": b1c,
            "w2_d": W2c,
            "b2_d": b2c,
        }
        for m in range(NCORES)
    ]
    return nc, in_maps


def kernel(x, edge_index, W1, b1, W2, b2):
    from concourse import bass_utils

    nc, in_maps = prepare(x, edge_index, W1, b1, W2, b2)

    # the axon/PJRT execute path occasionally hits a transient
    # device-unrecoverable error; retry a couple of times
    last_err = None
    for _attempt in range(3):
        try:
            res = bass_utils.run_bass_kernel_spmd(
                nc, in_maps, core_ids=list(range(NCORES))
            )
            break
        except Exception as e:  # noqa: BLE001
            last_err = e
            import time as _time

            _time.sleep(5.0)
    else:
        raise last_err
    out = np.concatenate(
        [res.results[m]["out_d"][:NPC] for m in range(NCORES)], axis=0
    )
    return out.astype(np.float32)


# revision 27
# speedup vs baseline: 4.2765x; 3.9119x over previous
"""GCN encoder (2-layer GCNConv) Trainium2 Bass kernel, 8-core SPMD.

out = A_hat @ relu(A_hat @ x @ W1 + b1) @ W2 + b2,  A_hat = D^-1/2 (A+I) D^-1/2

Strategy (1D graph partition by destination node):
 - nodes split into 8 contiguous ranges of 12500; each core owns its range's
   aggregations for both layers.
 - per core, non-self edges are sorted by (dest tile, source group) and
   padded into 128-edge chunks; all chunks of a group are fetched with a few
   large InstDMAGatherAnt instructions (GCH chunks each, single_packet=False,
   one SWDGE queue per group) instead of one indirect DMA per chunk -- the
   per-instruction ~1us Pool descriptor-emission fixed cost was the original
   bottleneck, and per-queue descriptor drain the next one.
 - dma_gather indices are int16, so sources are split into 4 groups of 25088
   rows.  The group of node v is (2*(half of v's local index) + v_core//4),
   which makes each group exactly one contiguous half of one of the two
   half-AllGather output buffers -- the layer-1 AllGather is split into two
   collectives so the first overlaps layer-1 tail compute and the second
   overlaps layer-2's gathers of the first half.  x is pre-permuted on the
   host into the same grouped order so both layers share one index table.
 - self loops never enter the gather path: layer 1 reads their rows with one
   dense DMA; layer 2 reads the SBUF-resident layer-1 outputs directly.
 - per chunk: build S[e, d] = (colw[e]==d) * norm[e] on DVE, accumulate
   aggT[F x 128d] += X_g^T @ S on PE (PSUM, f32).  Per tile: out[d, Fout] =
   aggT^T @ W + b via PE, relu on ACT, store.  All operands bf16.
"""

import numpy as np

N_NODES = 100000
N_EDGES = 640000
IN_CH = 128
OUT_CH = 64
HID = 128
NCORES = 8
NPC = N_NODES // NCORES          # 12500 nodes per core
P = 128
TILES = (NPC + P - 1) // P       # 98 dest tiles per core
PADN = TILES * P                 # 12544 padded rows per core slice
HTILES = (TILES + 1) // 2        # 49 tiles per collective half
HROWS = HTILES * P               # 6272 rows per half per core
NSEG = 4
GRP = 4 * HROWS                  # 25088 rows per gather group (int16-safe)
GCH = 16                         # chunks (x128 rows) per dma_gather
SCRATCH = 36864                  # SWDGE ring: 36864/16 = 2304 descriptors
DT = "bf16"

_CACHE = {}


def _group_of(v):
    """Gather group + row-within-group of source node v (vectorized)."""
    c = v // NPC
    l = v - c * NPC
    lh = l // HROWS
    g = 2 * lh + c // 4
    j = (c % 4) * HROWS + (l - lh * HROWS)
    return g, j


def _preprocess(edge_index):
    """Sort/partition/pad edges; build gather indices and S-build tables.

    Returns (sched, nseg_chunks, cw, nv, idx, perm) where
      sched: list of (tile, grp) per chunk in matmul order; grp==-1 is the
             per-tile self-loop chunk (always first).
      nseg_chunks[g]: chunks in group g (gather slot order).
      cw/nv: [NCORES, 128, C] f32 -- dest-in-tile / norm per chunk slot.
      idx: [NCORES, 128, cols] int16 packed gather indices (shared by both
             layers), columns ordered group-major.
      perm: x_grouped[perm] scatter map -- x_grouped[g*GRP+j] = x[v].
    """
    row = edge_index[0].astype(np.int64)
    col = edge_index[1].astype(np.int64)

    deg = (np.bincount(col, minlength=N_NODES) + 1.0).astype(np.float64)
    dinv = (1.0 / np.sqrt(deg)).astype(np.float32)
    norm = (dinv[row] * dinv[col]).astype(np.float32)

    core = col // NPC
    local = col - core * NPC
    tile = local // P
    colw = (local - tile * P).astype(np.float32)
    seg, jrow = _group_of(row)

    skey = (core * TILES + tile) * NSEG + seg
    order = np.argsort(skey, kind="stable")
    jrow_s = jrow[order]
    colw_s = colw[order]
    norm_s = norm[order]

    counts = np.bincount(skey, minlength=NCORES * TILES * NSEG).reshape(
        NCORES, TILES, NSEG
    )
    n_ts = ((counts + P - 1) // P).max(axis=0)  # [TILES, NSEG]

    sched = []
    chunk0 = np.zeros((TILES, NSEG), dtype=np.int64)
    selfc = np.zeros(TILES, dtype=np.int64)
    for t in range(TILES):
        selfc[t] = len(sched)
        sched.append((t, -1))
        for s in range(NSEG):
            chunk0[t, s] = len(sched)
            for _ in range(n_ts[t, s]):
                sched.append((t, s))
    C = len(sched)

    pos_in_seg = np.full(C, -1, dtype=np.int64)
    nseg_chunks = [0] * NSEG
    for c, (t, s) in enumerate(sched):
        if s >= 0:
            pos_in_seg[c] = nseg_chunks[s]
            nseg_chunks[s] += 1

    cw = np.zeros((NCORES, P, C), dtype=np.float32)
    nv = np.zeros((NCORES, P, C), dtype=np.float32)
    idx = [np.zeros((NCORES, nseg_chunks[s] * P), dtype=np.int16)
           for s in range(NSEG)]

    for t in range(TILES):
        c = selfc[t]
        nd = min(NPC - t * P, P)
        cw[:, :nd, c] = np.arange(nd, dtype=np.float32)
        for m in range(NCORES):
            v0 = m * NPC + t * P
            nv[m, :nd, c] = dinv[v0:v0 + nd] ** 2

    boundaries = np.searchsorted(
        skey[order], np.arange(NCORES * TILES * NSEG + 1), side="left"
    )
    for m in range(NCORES):
        for t in range(TILES):
            for s in range(NSEG):
                k = (m * TILES + t) * NSEG + s
                b0, b1 = boundaries[k], boundaries[k + 1]
                cnt = b1 - b0
                if cnt == 0:
                    continue
                c0 = chunk0[t, s]
                slot = np.arange(cnt)
                ch = slot // P
                part = slot % P
                cw[m, part, c0 + ch] = colw_s[b0:b1]
                nv[m, part, c0 + ch] = norm_s[b0:b1]
                flat = (pos_in_seg[c0] + ch) * P + part
                idx[s][m, flat] = jrow_s[b0:b1].astype(np.int16)

    def pack(seg_arrays):
        packed = []
        for m in range(NCORES):
            cols = []
            for s in range(NSEG):
                a = seg_arrays[s][m]
                t16 = a.reshape(-1, 16).T
                cols.append(np.tile(t16, (8, 1)))
            packed.append(np.concatenate(cols, axis=1))
        return np.ascontiguousarray(np.stack(packed))

    v = np.arange(N_NODES, dtype=np.int64)
    g_all, j_all = _group_of(v)
    perm = g_all * GRP + j_all  # x_grouped[perm[v]] = x[v]

    return sched, nseg_chunks, cw, nv, pack(idx), perm


def _build_module(sched, nseg_chunks, timing_mode=False, variant="full"):
    import concourse.bass as bass
    import concourse.bacc as bacc
    import concourse.tile as tile
    import concourse.mybir as mybir

    f32 = mybir.dt.float32
    i16 = mybir.dt.int16
    i32 = mybir.dt.int32
    dt = mybir.dt.bfloat16 if DT == "bf16" else f32

    C = len(sched)
    seg_col_off = np.concatenate(
        [[0], np.cumsum([nseg_chunks[s] * P // 16 for s in range(NSEG)])]
    ).astype(np.int64)
    idx_cols = int(seg_col_off[-1])
    n_gath = [(nseg_chunks[s] + GCH - 1) // GCH for s in range(NSEG)]

    ndev = 1 if timing_mode else NCORES
    nc = bacc.Bacc(
        "TRN2",
        target_bir_lowering=False,
        debug=False,
        num_devices=ndev,
        num_swdge_queues=4,
        dynamic_dma_scratch_size=SCRATCH,
    )

    xg_d = nc.dram_tensor("xg_d", [NSEG * GRP, IN_CH], dt,
                          kind="ExternalInput")
    xself_d = nc.dram_tensor("xself_d", [TILES, P, IN_CH], dt,
                             kind="ExternalInput")
    idx_d = nc.dram_tensor("idx_d", [P, idx_cols], i16, kind="ExternalInput")
    cw_d = nc.dram_tensor("cw_d", [P, C], f32, kind="ExternalInput")
    nv_d = nc.dram_tensor("nv_d", [P, C], f32, kind="ExternalInput")
    w1_d = nc.dram_tensor("w1_d", [IN_CH, HID], dt, kind="ExternalInput")
    b1_d = nc.dram_tensor("b1_d", [1, HID], dt, kind="ExternalInput")
    w2_d = nc.dram_tensor("w2_d", [HID, OUT_CH], dt, kind="ExternalInput")
    b2_d = nc.dram_tensor("b2_d", [1, OUT_CH], dt, kind="ExternalInput")

    h1_mine = nc.dram_tensor("h1_mine", [TILES, P, HID], dt)
    h1_lo = nc.dram_tensor("h1_lo", [NCORES * HROWS, HID], dt,
                           addr_space="Shared")
    h1_hi = nc.dram_tensor("h1_hi", [NCORES * HROWS, HID], dt,
                           addr_space="Shared")
    out_d = nc.dram_tensor("out_d", [PADN, OUT_CH], dt, kind="ExternalOutput")

    cfg = globals().get("_POOL_CFG") or {}
    with tile.TileContext(nc) as tc:
        with (
            tc.tile_pool(name="const", bufs=1) as cpool,
            tc.tile_pool(name="g0", bufs=cfg.get("SEG_BUFS", 4)) as gp0,
            tc.tile_pool(name="g1", bufs=cfg.get("SEG_BUFS", 4)) as gp1,
            tc.tile_pool(name="g2", bufs=cfg.get("SEG_BUFS", 4)) as gp2,
            tc.tile_pool(name="g3", bufs=cfg.get("SEG_BUFS", 4)) as gp3,
            tc.tile_pool(name="sel", bufs=cfg.get("SEL_BUFS", 10)) as spool,
            tc.tile_pool(name="out", bufs=cfg.get("OUT_BUFS", 8)) as opool,
            tc.tile_pool(name="psA", bufs=cfg.get("PSA_BUFS", 4), space="PSUM") as psA,
            tc.tile_pool(name="psB", bufs=cfg.get("PSB_BUFS", 3), space="PSUM") as psB,
        ):
            gpools = [gp0, gp1, gp2, gp3]

            iota_i = cpool.tile([P, P], i32)
            nc.gpsimd.iota(iota_i[:], pattern=[[1, P]], base=0,
                           channel_multiplier=0)
            iota_f = cpool.tile([P, P], dt)
            nc.vector.tensor_copy(out=iota_f[:], in_=iota_i[:])

            idx_s = cpool.tile([P, idx_cols], i16)
            nc.sync.dma_start(out=idx_s[:], in_=idx_d[:, :])
            cw_s = cpool.tile([P, C], f32)
            nc.sync.dma_start(out=cw_s[:], in_=cw_d[:, :])
            nv_s = cpool.tile([P, C], f32)
            nc.sync.dma_start(out=nv_s[:], in_=nv_d[:, :])

            w1_s = cpool.tile([IN_CH, HID], dt)
            nc.sync.dma_start(out=w1_s[:], in_=w1_d[:, :])
            b1_s = cpool.tile([1, HID], dt)
            nc.sync.dma_start(out=b1_s[:], in_=b1_d[:, :])
            w2_s = cpool.tile([HID, OUT_CH], dt)
            nc.sync.dma_start(out=w2_s[:], in_=w2_d[:, :])
            b2_s = cpool.tile([1, OUT_CH], dt)
            nc.sync.dma_start(out=b2_s[:], in_=b2_d[:, :])
            ones_s = cpool.tile([1, P], dt)
            nc.vector.memset(ones_s[:], 1.0)

            # layer-1 self rows (x slice, padded) and resident layer-1 output
            xself_s = cpool.tile([P, TILES, IN_CH], dt)
            nc.scalar.dma_start(
                out=xself_s[:, :, :],
                in_=xself_d[:, :, :].rearrange("t p f -> p t f"),
            )
            h1self_s = cpool.tile([P, TILES, HID], dt)

            # SWDGE queue must follow the tile framework's DMASW lane
            # round-robin (lane = emission_index % 8, so queue must be
            # emission_index % 4) or sem lanes get cross-queue updates
            swdge_ctr = [0]

            def layer(seg_srcs, self_tiles, w_s, b_s, fout, relu,
                      gathers_only=False, post_tile=None,
                      group_phases=None, mid_hook=None):
                seg_tiles = [[None] * n_gath[s] for s in range(NSEG)]

                def emit_gathers(groups):
                    for g in range(max(n_gath)):
                        for s in groups:
                            if g >= n_gath[s]:
                                continue
                            k = min(GCH, nseg_chunks[s] - g * GCH)
                            xg = gpools[s].tile([P, GCH, IN_CH], dt, tag="xg")
                            nc.gpsimd.dma_gather(
                                xg[:, 0:k, :],
                                seg_srcs[s],
                                idx_s[:, seg_col_off[s] + g * GCH * 8:
                                      seg_col_off[s] + (g * GCH + k) * 8],
                                k * P,
                                k * P,
                                IN_CH,
                                single_packet=False,
                                queue_num=swdge_ctr[0] % 4,
                            )
                            swdge_ctr[0] += 1
                            seg_tiles[s][g] = xg

                if group_phases is None:
                    group_phases = [list(range(NSEG))]
                emit_gathers(group_phases[0])
                if mid_hook is not None:
                    mid_hook()
                for ph in group_phases[1:]:
                    emit_gathers(ph)

                if gathers_only:
                    return
                pos = [0] * NSEG
                c = 0
                for t in range(TILES):
                    aggT = psA.tile([P, P], f32, space="PSUM", tag="aggT")
                    nch = 1
                    while c + nch < C and sched[c + nch][0] == t:
                        nch += 1
                    for j in range(nch):
                        tt, s = sched[c + j]
                        if s < 0:
                            lhsT = self_tiles[:, t, :]
                        else:
                            p = pos[s]
                            pos[s] += 1
                            lhsT = seg_tiles[s][p // GCH][:, p % GCH, :]
                        S = spool.tile([P, P], dt, tag="S")
                        nc.vector.tensor_scalar(
                            out=S[:],
                            in0=iota_f[:],
                            scalar1=cw_s[:, c + j:c + j + 1],
                            scalar2=nv_s[:, c + j:c + j + 1],
                            op0=mybir.AluOpType.is_equal,
                            op1=mybir.AluOpType.mult,
                        )
                        nc.tensor.matmul(
                            out=aggT[:],
                            lhsT=lhsT,
                            rhs=S[:],
                            start=(j == 0),
                            stop=(j == nch - 1),
                        )
                    c += nch
                    aggT_s = spool.tile([P, P], dt, tag="aggTs")
                    nc.scalar.copy(out=aggT_s[:], in_=aggT[:])
                    h_ps = psB.tile([P, fout], f32, space="PSUM", tag="h")
                    nc.tensor.matmul(
                        out=h_ps[:], lhsT=ones_s[:], rhs=b_s[:],
                        start=True, stop=False,
                    )
                    nc.tensor.matmul(
                        out=h_ps[:], lhsT=aggT_s[:], rhs=w_s[:],
                        start=False, stop=True,
                    )
                    if relu:
                        nc.scalar.activation(
                            out=h1self_s[:, t, :],
                            in_=h_ps[:],
                            func=mybir.ActivationFunctionType.Relu,
                        )
                        nc.sync.dma_start(out=h1_mine[t, :, :],
                                          in_=h1self_s[:, t, :])
                    else:
                        h_sb = opool.tile([P, fout], dt, tag="ho")
                        nc.vector.tensor_copy(out=h_sb[:], in_=h_ps[:])
                        nc.sync.dma_start(
                            out=out_d[t * P:(t + 1) * P, :], in_=h_sb[:]
                        )
                    if post_tile is not None:
                        post_tile(t)

            def emit_ag(lo):
                nc.gpsimd.collective_compute(
                    "AllGather",
                    mybir.AluOpType.bypass,
                    replica_groups=[list(range(NCORES))],
                    ins=[h1_mine[0:HTILES, :, :].opt() if lo
                         else h1_mine[HTILES:TILES, :, :].opt()],
                    outs=[h1_lo[:, :].opt() if lo else h1_hi[:, :].opt()],
                )

            do_coll = (not timing_mode) and variant in ("full", "coll")

            def post_tile(t):
                if not do_coll:
                    return
                if t == HTILES - 1:
                    emit_ag(lo=True)

            go = variant == "gathers"
            if variant != "coll":
                layer([xg_d[s * GRP:(s + 1) * GRP, :] for s in range(NSEG)],
                      xself_s, w1_s, b1_s, HID, relu=True, gathers_only=go,
                      post_tile=post_tile)
            else:
                t0 = opool.tile([P, HID], dt, tag="ho")
                nc.vector.memset(t0[:], 0.0)
                for t in range(TILES):
                    nc.sync.dma_start(out=h1_mine[t, :, :], in_=t0[:])
                emit_ag(lo=True)
                emit_ag(lo=False)

            if do_coll and variant != "coll":
                emit_ag(lo=False)
            if variant not in ("l1", "coll"):
                # lo-group gathers depend only on AG_lo's output, so they
                # overlap AG_hi (collectives complete on their own lane)
                layer([h1_lo[0:GRP, :], h1_lo[GRP:2 * GRP, :],
                       h1_hi[0:GRP, :], h1_hi[GRP:2 * GRP, :]],
                      h1self_s, w2_s, b2_s, OUT_CH, relu=False,
                      gathers_only=go)

    nc.compile()
    return nc


def _np_dt():
    if DT == "bf16":
        import ml_dtypes

        return np.dtype(ml_dtypes.bfloat16)
    return np.dtype(np.float32)


def prepare(x, edge_index, W1, b1, W2, b2):
    """Compile (cached) and build per-core input maps."""
    edge_index = np.asarray(edge_index)
    key = hash(edge_index.tobytes())
    if key not in _CACHE:
        sched, nseg_chunks, cw, nv, idx, perm = _preprocess(edge_index)
        nc = _build_module(sched, nseg_chunks)
        _CACHE.clear()
        _CACHE[key] = (nc, sched, nseg_chunks, cw, nv, idx, perm)
    nc, sched, nseg_chunks, cw, nv, idx, perm = _CACHE[key]

    dt = _np_dt()
    x = np.asarray(x, dtype=np.float32).astype(dt)
    xg = np.zeros((NSEG * GRP, IN_CH), dtype=dt)
    xg[perm] = x
    xself = np.zeros((NCORES, TILES, P, IN_CH), dtype=dt)
    for m in range(NCORES):
        xself[m].reshape(PADN, IN_CH)[:NPC] = x[m * NPC:(m + 1) * NPC]
    W1c = np.ascontiguousarray(np.asarray(W1, dtype=np.float32).astype(dt))
    b1c = np.asarray(b1, dtype=np.float32).astype(dt).reshape(1, HID)
    W2c = np.ascontiguousarray(np.asarray(W2, dtype=np.float32).astype(dt))
    b2c = np.asarray(b2, dtype=np.float32).astype(dt).reshape(1, OUT_CH)

    in_maps = [
        {
            "xg_d": xg,
            "xself_d": xself[m],
            "idx_d": idx[m],
            "cw_d": np.ascontiguousarray(cw[m]),
            "nv_d": np.ascontiguousarray(nv[m]),
            "w1_d": W1c,
            "b1_d": b1c,
            "w2_d": W2c,
            "b2_d": b2c,
        }
        for m in range(NCORES)
    ]
    return nc, in_maps


def kernel(x, edge_index, W1, b1, W2, b2):
    from concourse import bass_utils

    nc, in_maps = prepare(x, edge_index, W1, b1, W2, b2)

    # the axon/PJRT execute path occasionally hits a transient
    # device-unrecoverable error; retry a couple of times
    last_err = None
    for _attempt in range(3):
        try:
            res = bass_utils.run_bass_kernel_spmd(
                nc, in_maps, core_ids=list(range(NCORES))
            )
            break
        except Exception as e:  # noqa: BLE001
            last_err = e
            import time as _time

            _time.sleep(5.0)
    else:
        raise last_err
    out = np.concatenate(
        [res.results[m]["out_d"][:NPC] for m in range(NCORES)], axis=0
    )
    return out.astype(np.float32)
